# revision 18
# baseline (speedup 1.0000x reference)
"""BinaryTreeComposer (tree-LSTM node composition) on 8 TRN2 NeuronCores.

Strategy: output-dim (row) tensor-parallel shard of every 2048x2048 weight.
Core d owns rows [256*d, 256*(d+1)) of all 11 matrices.

Per-core dataflow (J = 256 output shard, K chunks of 128 partitions):
  stage1: m_pre[2,J]  = [lh|S ; rh|S](4096) @ [Wh_s|Us_s]^T    (32 matmuls, N=256)
  stage2: p[2,J]      = [lh ; rh](2048)     @ ma_s^T           (16 matmuls, N=256)
          (alpha scaling hoisted: (a*lh)@W = a*(lh@W))
  m = tanh(m_pre + b1);  e_part[2,1] = rowdot(m, w_s)
  ONE AllGather of [p | e_part]  ->  [16, 257] on every core
  alpha from gathered e partials; la/ra = tanh(alpha * p_full + ma_b)
  PE-transpose la/ra into [128, 16] stationary layout
  gates: z[1,1024] = [la;ra](4096) @ [G_l|G_r]^T for 4 gates   (64 matmuls, N=512)
  c = i*u + lf*lc_s + rf*rc_s ; h = tanh(c); out = [c_s | h_s] (1,512)

All matmul operands bf16 (full PE rate, halves HBM traffic); everything
else f32.  PSUM accumulation is f32.
"""

import os
import sys

import numpy as np

for _p in ("/opt/trn_rl_repo",):
    if _p not in sys.path and os.path.isdir(_p):
        sys.path.insert(0, _p)

from ml_dtypes import bfloat16  # noqa: E402

MEM = 2048
NCORES = 8
J = MEM // NCORES  # 256
KP = 128  # contraction chunk (partition count)
NCH1 = (2 * MEM) // KP  # 32 chunks for stage1 (concat K=4096)
NCH2 = MEM // KP  # 16 chunks for stage2
NCHG = (2 * MEM) // KP  # 32 chunks for gates

# matmul dtype: "bf16" (fast; DMA volume halved) or "f32" (4x slower PE, 2x DMA)
MM_DTYPE = os.environ.get("BTC_MM_DTYPE", "bf16")

_COMPILED = {}
LAST_RESULT = None  # BassKernelResults of the most recent run (for profiling)


def _ensure_ntff_hook():
    """Make trace=True work under axon: register the NTFF profile hook
    (the image's antenv lacks axon_hooks) and de-fang upload_artifacts
    (no egress in this container)."""
    import types

    try:
        import antenv  # noqa: F401

        if "antenv.axon_hooks" not in sys.modules:
            from trn_agent_boot.trn_boot import _ntff_profile_via_ctypes

            hook = _ntff_profile_via_ctypes("/opt/axon/libaxon_pjrt.so")
            mod = types.ModuleType("antenv.axon_hooks")
            state = {"hook": hook}
            mod.get_axon_ntff_profile_hook = lambda: state["hook"]
            mod.set_axon_ntff_profile_hook = lambda h: state.update(hook=h)
            sys.modules["antenv.axon_hooks"] = mod
            antenv.axon_hooks = mod
    except Exception:
        pass
    try:
        from concourse import bass_utils

        orig = bass_utils.upload_artifacts
        if not getattr(orig, "_btc_safe", False):

            def safe_upload(tmpdir):
                try:
                    return orig(tmpdir)
                except Exception:
                    return str(tmpdir)

            safe_upload._btc_safe = True
            bass_utils.upload_artifacts = safe_upload
    except Exception:
        pass


def _np_mm_dtype():
    return bfloat16 if MM_DTYPE == "bf16" else np.float32


def _build_nc():
    from contextlib import ExitStack

    import concourse.bass as bass
    import concourse.tile as tile
    from concourse import bacc, mybir

    f32 = mybir.dt.float32
    mdt = mybir.dt.bfloat16 if MM_DTYPE == "bf16" else mybir.dt.float32
    Tanh = mybir.ActivationFunctionType.Tanh
    Sigmoid = mybir.ActivationFunctionType.Sigmoid
    add = mybir.AluOpType.add
    mult = mybir.AluOpType.mult
    bypass = mybir.AluOpType.bypass

    nc = bacc.Bacc(
        "TRN2",
        target_bir_lowering=False,
        debug=False,
        num_devices=NCORES,
    )

    # ---- kernel I/O (per-core data supplied via in_maps) ----
    x1sp_d = nc.dram_tensor("x1sp", [KP, NCH1, 2], mdt, kind="ExternalInput")
    w1m_d = nc.dram_tensor("w1m", [NCH1, KP, J], mdt, kind="ExternalInput")
    w2m_d = nc.dram_tensor("w2m", [NCH2, KP, J], mdt, kind="ExternalInput")
    wg_d = nc.dram_tensor("wg", [NCHG, KP, 4 * J], mdt, kind="ExternalInput")
    b1_d = nc.dram_tensor("b1", [2, J], f32, kind="ExternalInput")
    wr_d = nc.dram_tensor("wr", [2, J], f32, kind="ExternalInput")
    brep_d = nc.dram_tensor("brep", [2 * NCORES, J], f32, kind="ExternalInput")
    emask_d = nc.dram_tensor("emask", [2 * NCORES, 2 * NCORES], f32, kind="ExternalInput")
    ident_d = nc.dram_tensor("ident", [2 * NCORES, 2 * NCORES], mdt, kind="ExternalInput")
    bg_d = nc.dram_tensor("bg", [1, 4 * J], f32, kind="ExternalInput")
    lcrc_d = nc.dram_tensor("lcrc", [1, 2 * J], f32, kind="ExternalInput")
    out_d = nc.dram_tensor("out", [1, 2 * J], f32, kind="ExternalOutput")

    R2 = 2 * NCORES  # 16 gathered rows
    trunc = int(os.environ.get("BTC_TRUNC", "0"))

    def body(tc, sb, ps, dram):
        # ---- SBUF loads ----
        x1t = sb.tile([KP, NCH1, 2], mdt, tag="x1t")
        nc.sync.dma_start(x1t[:], x1sp_d.ap())

        b1t = sb.tile([2, J], f32, tag="b1t")
        nc.sync.dma_start(b1t[:], b1_d.ap())
        wrt = sb.tile([2, J], f32, tag="wrt")
        nc.sync.dma_start(wrt[:], wr_d.ap())
        brept = sb.tile([R2, J], f32, tag="brept")
        nc.sync.dma_start(brept[:], brep_d.ap())
        emaskt = sb.tile([R2, R2], f32, tag="emaskt")
        nc.sync.dma_start(emaskt[:], emask_d.ap())
        identt = sb.tile([R2, R2], mdt, tag="identt")
        nc.sync.dma_start(identt[:], ident_d.ap())
        bgt = sb.tile([1, 4 * J], f32, tag="bgt")
        nc.sync.dma_start(bgt[:], bg_d.ap())
        lcrct = sb.tile([1, 2 * J], f32, tag="lcrct")
        nc.sync.dma_start(lcrct[:], lcrc_d.ap())

        # weight streams: [chunk, 128, N] in DRAM -> [128, chunk, N] SBUF tiles
        # (one DMA per tile; partition-contiguous granules of N*elt bytes)
        CPB1 = 8  # chunks per DMA for stage1/2 (8 * 64KB = 512KB bf16)
        w1tiles = []
        for b in range(NCH1 // CPB1):
            t = sb.tile([KP, CPB1, J], mdt, tag=f"w1_{b}")
            src = w1m_d.ap()[b * CPB1 : (b + 1) * CPB1, :, :].transpose([1, 0, 2])
            nc.sync.dma_start(t[:], src)
            w1tiles.append(t)
        w2tiles = []
        for b in range(NCH2 // CPB1):
            t = sb.tile([KP, CPB1, J], mdt, tag=f"w2_{b}")
            src = w2m_d.ap()[b * CPB1 : (b + 1) * CPB1, :, :].transpose([1, 0, 2])
            nc.sync.dma_start(t[:], src)
            w2tiles.append(t)
        CPBG = 4  # gate chunks per DMA (4 * 256KB = 1MB bf16)
        wgtiles = []
        for b in range(NCHG // CPBG):
            t = sb.tile([KP, CPBG, 4 * J], mdt, tag=f"wg_{b}")
            src = wg_d.ap()[b * CPBG : (b + 1) * CPBG, :, :].transpose([1, 0, 2])
            nc.sync.dma_start(t[:], src)
            wgtiles.append(t)

        # ---- stage 1 + stage 2 matmuls ----
        psum1 = ps.tile([2, J], f32, tag="psum1")
        for c in range(NCH1):
            nc.tensor.matmul(
                psum1[:],
                x1t[:, c, :],
                w1tiles[c // CPB1][:, c % CPB1, :],
                start=(c == 0),
                stop=(c == NCH1 - 1),
            )
        psum2 = ps.tile([2, J], f32, tag="psum2")
        for c in range(NCH2):
            nc.tensor.matmul(
                psum2[:],
                x1t[:, c, :],
                w2tiles[c // CPB1][:, c % CPB1, :],
                start=(c == 0),
                stop=(c == NCH2 - 1),
            )

        def dbg_out(row0_ap, row1_ap):
            """Debug escape: write two [1,J] APs into out and stop."""
            dbg = sb.tile([1, 2 * J], f32, tag="dbg")
            nc.vector.tensor_copy(dbg[:, 0:J], row0_ap)
            nc.vector.tensor_copy(dbg[:, J : 2 * J], row1_ap)
            nc.sync.dma_start(out_d.ap(), dbg[:])

        if trunc == 1:  # stage1/2 matmuls + weight DMAs only
            dbg_out(psum1[0:1, :], psum2[0:1, :])
            return
        # ---- stage-1 tail: m = tanh(pre + b1); e_part = rowdot(m, w) ----
        pre1 = sb.tile([2, J], f32, tag="pre1")
        nc.vector.tensor_tensor(pre1[:], psum1[:], b1t[:], add)
        s1 = sb.tile([2, J + 1], f32, tag="s1")
        nc.vector.tensor_copy(s1[:, 0:J], psum2[:])
        if trunc == 11:  # tt add + psum2 copy only
            dbg_out(s1[0:1, 0:J], pre1[0:1, :])
            return
        m = sb.tile([2, J], f32, tag="m")
        nc.scalar.activation(m[:], pre1[:], Tanh)
        if trunc == 115:  # + ACT tanh, no ttr
            dbg_out(s1[0:1, 0:J], m[0:1, :])
            return

        scr = sb.tile([2, J], f32, tag="scr")
        nc.vector.tensor_tensor(scr[:], m[:], wrt[:], mult)
        nc.vector.tensor_reduce(s1[:, J : J + 1], scr[:], mybir.AxisListType.X, add)

        if trunc == 12:  # stage-1 tail, no AG
            dbg_out(s1[0:1, 0:J], scr[0:1, :])
            return

        # ---- the one AllGather: [2, 257] -> [16, 257] ----
        agin = dram.tile([2, J + 1], f32)
        agout = dram.tile([R2, J + 1], f32)
        nc.sync.dma_start(agin[:], s1[:])
        nc.gpsimd.collective_compute(
            "AllGather",
            bypass,
            replica_groups=[list(range(NCORES))],
            ins=[agin.opt()],
            outs=[agout.opt()],
        )

        # gathered p -> [16, 256]; e column broadcast -> [16, 16]
        pg = sb.tile([R2, J], f32, tag="pg")
        nc.sync.dma_start(pg[:], agout[:, 0:J])
        ebt = sb.tile([R2, R2], f32, tag="ebt")
        esrc = agout[:, J : J + 1].transpose([1, 0]).partition_broadcast(R2)
        nc.gpsimd.dma_start(ebt[:], esrc)

        if trunc == 13:  # + AG + post-AG DMAs, no alpha math
            dbg = sb.tile([1, 2 * J], f32, tag="dbg")
            nc.vector.tensor_copy(dbg[:, 0:J], pg[0:1, :])
            nc.vector.tensor_copy(dbg[:, J : J + 16], ebt[0:1, :])
            nc.sync.dma_start(out_d.ap(), dbg[:])
            return

        # ---- alphas on all 16 rows ----
        den = sb.tile([R2, 1], f32, tag="den")
        nc.vector.tensor_reduce(den[:], ebt[:], mybir.AxisListType.X, add)
        scr16 = sb.tile([R2, R2], f32, tag="scr16")
        esel = sb.tile([R2, 1], f32, tag="esel")
        nc.vector.tensor_tensor(scr16[:], ebt[:], emaskt[:], mult)
        nc.vector.tensor_reduce(esel[:], scr16[:], mybir.AxisListType.X, add)
        denr = sb.tile([R2, 1], f32, tag="denr")
        nc.vector.reciprocal(denr[:], den[:])
        alpha = sb.tile([R2, 1], f32, tag="alpha")
        nc.vector.tensor_tensor(alpha[:], esel[:], denr[:], mult)

        if trunc == 2:  # through AG + alpha
            dbg = sb.tile([1, 2 * J], f32, tag="dbg")
            nc.vector.tensor_copy(dbg[:, 0:J], pg[0:1, :])
            nc.vector.tensor_copy(dbg[:, J : J + 1], alpha[0:1, :])
            nc.vector.tensor_copy(dbg[:, J + 1 : J + 17], ebt[0:1, :])
            nc.sync.dma_start(out_d.ap(), dbg[:])
            return
        # ---- la/ra: tanh(alpha * p + ma_b), in gathered [16, 256] layout ----
        actin = sb.tile([R2, J], f32, tag="actin")
        nc.vector.scalar_tensor_tensor(
            actin[:], pg[:], alpha[:], brept[:], mult, add
        )
        laR = sb.tile([R2, J], mdt, tag="laR")
        nc.scalar.activation(laR[:], actin[:], Tanh)

        # ---- PE transpose into stationary layout: T0/T1 [128, 16] ----
        Ts = []
        for h in range(2):
            pt = ps.tile([KP, R2], mdt, tag=f"psT{h}")
            nc.tensor.transpose(pt[:], laR[:, h * KP : (h + 1) * KP], identt[:])
            t = sb.tile([KP, R2], mdt, tag=f"T{h}")
            nc.vector.tensor_copy(t[:], pt[:])
            Ts.append(t)

        if trunc == 3:  # through PE transposes
            dbg = sb.tile([1, 2 * J], f32, tag="dbg")
            nc.vector.tensor_copy(dbg[:, 0:R2], Ts[0][0:1, :])
            nc.vector.tensor_copy(dbg[:, R2 : 2 * R2], Ts[1][0:1, :])
            nc.sync.dma_start(out_d.ap(), dbg[:])
            return

        # stationary column for xcat chunk c (c<16: la chunk c; else ra chunk c-16)
        def xcat_col(c):
            cc = c % NCH2
            col = 2 * (cc // 2) + (0 if c < NCH2 else 1)
            return Ts[cc % 2][:, col : col + 1]

        # ---- gate matmuls: two N=512 accumulation groups ----
        psgA = ps.tile([1, 2 * J], f32, tag="psgA")
        psgB = ps.tile([1, 2 * J], f32, tag="psgB")
        for c in range(NCHG):
            lhs = xcat_col(c)
            wt = wgtiles[c // CPBG]
            nc.tensor.matmul(
                psgA[:],
                lhs,
                wt[:, c % CPBG, 0 : 2 * J],
                start=(c == 0),
                stop=(c == NCHG - 1),
            )
            nc.tensor.matmul(
                psgB[:],
                lhs,
                wt[:, c % CPBG, 2 * J : 4 * J],
                start=(c == 0),
                stop=(c == NCHG - 1),
            )

        # ---- gate tail ----
        zA = sb.tile([1, 2 * J], f32, tag="zA")
        nc.vector.tensor_tensor(zA[:], psgA[:], bgt[:, 0 : 2 * J], add)
        zB = sb.tile([1, 2 * J], f32, tag="zB")
        nc.vector.tensor_tensor(zB[:], psgB[:], bgt[:, 2 * J : 4 * J], add)
        sA = sb.tile([1, 2 * J], f32, tag="sA")  # [i | lf]
        nc.scalar.activation(sA[:], zA[:], Sigmoid)
        sB = sb.tile([1, 2 * J], f32, tag="sB")  # [rf | u]
        nc.scalar.activation(sB[:, 0:J], zB[:, 0:J], Sigmoid)
        nc.scalar.activation(sB[:, J : 2 * J], zB[:, J : 2 * J], Tanh)

        co = sb.tile([1, 2 * J], f32, tag="co")
        t1 = sb.tile([1, J], f32, tag="t1")
        nc.vector.tensor_tensor(t1[:], sA[:, 0:J], sB[:, J : 2 * J], mult)  # i*u
        t2 = sb.tile([1, J], f32, tag="t2")
        nc.vector.tensor_tensor(t2[:], sA[:, J : 2 * J], lcrct[:, 0:J], mult)  # lf*lc
        t3 = sb.tile([1, J], f32, tag="t3")
        nc.vector.tensor_tensor(t3[:], sB[:, 0:J], lcrct[:, J : 2 * J], mult)  # rf*rc
        t4 = sb.tile([1, J], f32, tag="t4")
        nc.vector.tensor_tensor(t4[:], t1[:], t2[:], add)
        nc.vector.tensor_tensor(co[:, 0:J], t4[:], t3[:], add)  # c
        nc.scalar.activation(co[:, J : 2 * J], co[:, 0:J], Tanh)  # h

        nc.sync.dma_start(out_d.ap(), co[:])

    with ExitStack() as ctx:
        tc = ctx.enter_context(tile.TileContext(nc))
        sb = ctx.enter_context(tc.tile_pool(name="sb", bufs=1))
        ps = ctx.enter_context(tc.tile_pool(name="ps", bufs=1, space="PSUM"))
        dram = ctx.enter_context(tc.tile_pool(name="dram", bufs=1, space="DRAM"))
        body(tc, sb, ps, dram)

    nc.compile()
    return nc


def _shard_inputs(inp):
    mdt = _np_mm_dtype()
    f32 = np.float32

    def a(x):
        return np.asarray(x, dtype=np.float32)

    lh, rh, S = a(inp["lh"])[0], a(inp["rh"])[0], a(inp["S"])[0]
    lc, rc, w = a(inp["lc"])[0], a(inp["rc"])[0], a(inp["w"])[0]

    # stationary x spread: [128, 32, 2]; cat = [lh|S] col0, [rh|S] col1
    xl = np.concatenate([lh, S]).reshape(NCH1, KP).T  # [128, 32]
    xr = np.concatenate([rh, S]).reshape(NCH1, KP).T
    x1sp = np.stack([xl, xr], axis=-1).astype(mdt)  # [128, 32, 2]

    R2 = 2 * NCORES
    emask = np.zeros((R2, R2), np.float32)
    for r in range(R2):
        emask[r, r % 2 :: 2] = 1.0
    ident = np.eye(R2, dtype=mdt)
    brep = np.repeat(a(inp["ma_b"]).reshape(NCORES, J), 2, axis=0).astype(f32)

    in_maps = []
    for d in range(NCORES):
        sl = slice(d * J, (d + 1) * J)
        w1m = np.concatenate(
            [a(inp["Wh_w"])[sl].T, a(inp["Us_w"])[sl].T], axis=0
        ).reshape(NCH1, KP, J).astype(mdt)
        w2m = a(inp["ma_w"])[sl].T.reshape(NCH2, KP, J).copy().astype(mdt)
        gs = []
        for l, r in (("ilh", "irh"), ("lflh", "lfrh"), ("rflh", "rfrh"), ("ulh", "urh")):
            gs.append(
                np.concatenate([a(inp[l + "_w"])[sl].T, a(inp[r + "_w"])[sl].T], axis=0)
            )  # [4096, 256]
        wg = (
            np.stack(gs, axis=1).reshape(2 * MEM, 4 * J).reshape(NCHG, KP, 4 * J)
        ).astype(mdt)
        b1 = np.tile((a(inp["Wh_b"]) + a(inp["Us_b"]))[sl], (2, 1)).astype(f32)
        wr = np.tile(w[sl], (2, 1)).astype(f32)
        bg = np.concatenate(
            [
                (a(inp["ilh_b"]) + a(inp["irh_b"]))[sl],
                (a(inp["lflh_b"]) + a(inp["lfrh_b"]))[sl],
                (a(inp["rflh_b"]) + a(inp["rfrh_b"]))[sl],
                (a(inp["ulh_b"]) + a(inp["urh_b"]))[sl],
            ]
        ).reshape(1, 4 * J).astype(f32)
        lcrc = np.concatenate([lc[sl], rc[sl]]).reshape(1, 2 * J).astype(f32)
        in_maps.append(
            {
                "x1sp": x1sp,
                "w1m": w1m,
                "w2m": w2m,
                "wg": wg,
                "b1": b1,
                "wr": wr,
                "brep": brep,
                "emask": emask,
                "ident": ident,
                "bg": bg,
                "lcrc": lcrc,
            }
        )
    return in_maps


def kernel(**inputs):
    global LAST_RESULT
    _ensure_ntff_hook()
    from concourse.bass_utils import run_bass_kernel_spmd

    key = MM_DTYPE
    if key not in _COMPILED:
        _COMPILED[key] = _build_nc()
    nc = _COMPILED[key]

    in_maps = _shard_inputs(inputs)
    res = run_bass_kernel_spmd(nc, in_maps, list(range(NCORES)))
    LAST_RESULT = res
    c = np.concatenate([res.results[d]["out"][0, 0:J] for d in range(NCORES)])
    h = np.concatenate([res.results[d]["out"][0, J : 2 * J] for d in range(NCORES)])
    return (c.reshape(1, MEM).astype(np.float32), h.reshape(1, MEM).astype(np.float32))


# revision 19
# speedup vs baseline: 1.6813x; 1.6813x over previous
"""BinaryTreeComposer (tree-LSTM node composition) on 8 TRN2 NeuronCores.

Strategy: output-dim (row) tensor-parallel shard of every 2048x2048 weight.
Core d owns rows [256*d, 256*(d+1)) of all 11 matrices.

Two collective-free launches (measured: each ncfw collective costs ~20us
on this runtime plus ~40us launch skew absorbed by the first one, so a
single-launch all-gather design floors at ~110us):

  Launch A (per core, J=256 shard):
    m_pre[2,J] = [lh|S ; rh|S](4096) @ [Wh_s|Us_s]^T   (32 matmuls, N=256)
    p[2,J]     = [lh ; rh](2048)     @ ma_s^T          (16 matmuls, N=256)
      (alpha scaling hoisted past the GEMV: (a*lh)@W = a*(lh@W))
    m = tanh(m_pre + b1); e_part[2,1] = rowdot(m, w_s)
    out s1[2, J+1] = [p | e_part]

  Host: ag[16, J+1] = concat(s1 over cores)   (data movement only)

  Launch B (per core):
    alpha[16,1] from the 16 gathered e partials (mask-reduce trick)
    la/ra[16,J] = tanh(alpha * p_gathered + ma_b)
    PE-transpose la/ra into [128,16] stationary layout
    z[1,1024] = [la;ra](4096) @ [G_l|G_r]^T for 4 gates (64 matmuls, N=512)
    c = i*u + lf*lc_s + rf*rc_s ; h = tanh(c); out = [c_s | h_s]

All matmul operands bf16 (full PE rate, halves HBM traffic); everything
else f32; PSUM accumulation f32.
"""

import os
import sys

import numpy as np

for _p in ("/opt/trn_rl_repo",):
    if _p not in sys.path and os.path.isdir(_p):
        sys.path.insert(0, _p)

from ml_dtypes import bfloat16  # noqa: E402

MEM = 2048
NCORES = 8
J = MEM // NCORES  # 256
KP = 128  # contraction chunk (partition count)
NCH1 = (2 * MEM) // KP  # 32 chunks for stage1 (concat K=4096)
NCH2 = MEM // KP  # 16 chunks for stage2
NCHG = (2 * MEM) // KP  # 32 chunks for gates
R2 = 2 * NCORES  # 16 gathered rows

# matmul dtype: "bf16" (fast; DMA volume halved) or "f32" (4x slower PE, 2x DMA)
MM_DTYPE = os.environ.get("BTC_MM_DTYPE", "bf16")

_COMPILED = {}
LAST_RESULTS = []  # BassKernelResults of the most recent run (for profiling)


def _ensure_ntff_hook():
    """Make trace=True work under axon: register the NTFF profile hook
    (the image's antenv lacks axon_hooks) and de-fang upload_artifacts
    (no egress in this container)."""
    import types

    try:
        import antenv  # noqa: F401

        if "antenv.axon_hooks" not in sys.modules:
            from trn_agent_boot.trn_boot import _ntff_profile_via_ctypes

            hook = _ntff_profile_via_ctypes("/opt/axon/libaxon_pjrt.so")
            mod = types.ModuleType("antenv.axon_hooks")
            state = {"hook": hook}
            mod.get_axon_ntff_profile_hook = lambda: state["hook"]
            mod.set_axon_ntff_profile_hook = lambda h: state.update(hook=h)
            sys.modules["antenv.axon_hooks"] = mod
            antenv.axon_hooks = mod
    except Exception:
        pass
    try:
        from concourse import bass_utils

        orig = bass_utils.upload_artifacts
        if not getattr(orig, "_btc_safe", False):

            def safe_upload(tmpdir):
                try:
                    return orig(tmpdir)
                except Exception:
                    return str(tmpdir)

            safe_upload._btc_safe = True
            bass_utils.upload_artifacts = safe_upload
    except Exception:
        pass


def _np_mm_dtype():
    return bfloat16 if MM_DTYPE == "bf16" else np.float32


def _mk_nc():
    from concourse import bacc

    return bacc.Bacc(
        "TRN2", target_bir_lowering=False, debug=False, num_devices=NCORES
    )


def _ctx_pools(nc):
    from contextlib import ExitStack

    import concourse.tile as tile

    ctx = ExitStack()
    tc = ctx.enter_context(tile.TileContext(nc))
    sb = ctx.enter_context(tc.tile_pool(name="sb", bufs=1))
    ps = ctx.enter_context(tc.tile_pool(name="ps", bufs=1, space="PSUM"))
    return ctx, tc, sb, ps


def _build_a():
    """Stage 1 + 2: row-sharded attention GEMVs -> s1 = [p | e_part]."""
    from concourse import mybir

    f32 = mybir.dt.float32
    mdt = mybir.dt.bfloat16 if MM_DTYPE == "bf16" else mybir.dt.float32
    Tanh = mybir.ActivationFunctionType.Tanh
    add = mybir.AluOpType.add
    mult = mybir.AluOpType.mult

    nc = _mk_nc()
    x1sp_d = nc.dram_tensor("x1sp", [KP, NCH1, 2], mdt, kind="ExternalInput")
    w1m_d = nc.dram_tensor("w1m", [NCH1, KP, J], mdt, kind="ExternalInput")
    w2m_d = nc.dram_tensor("w2m", [NCH2, KP, J], mdt, kind="ExternalInput")
    b1_d = nc.dram_tensor("b1", [2, J], f32, kind="ExternalInput")
    wr_d = nc.dram_tensor("wr", [2, J], f32, kind="ExternalInput")
    s1_d = nc.dram_tensor("s1", [2, J + 1], f32, kind="ExternalOutput")

    ctx, tc, sb, ps = _ctx_pools(nc)
    with ctx:
        # weight streams first on the sync HWDGE ring (critical path)
        CPB1 = 8  # chunks per DMA (8 * 64KB bf16 = 512KB)
        w1tiles = []
        for b in range(NCH1 // CPB1):
            t = sb.tile([KP, CPB1, J], mdt, tag=f"w1_{b}")
            nc.sync.dma_start(
                t[:], w1m_d.ap()[b * CPB1 : (b + 1) * CPB1].transpose([1, 0, 2])
            )
            w1tiles.append(t)
        w2tiles = []
        for b in range(NCH2 // CPB1):
            t = sb.tile([KP, CPB1, J], mdt, tag=f"w2_{b}")
            nc.sync.dma_start(
                t[:], w2m_d.ap()[b * CPB1 : (b + 1) * CPB1].transpose([1, 0, 2])
            )
            w2tiles.append(t)
        # small inputs on the scalar HWDGE ring (independent of weight stream)
        x1t = sb.tile([KP, NCH1, 2], mdt, tag="x1t")
        nc.scalar.dma_start(x1t[:], x1sp_d.ap())
        b1t = sb.tile([2, J], f32, tag="b1t")
        nc.scalar.dma_start(b1t[:], b1_d.ap())
        wrt = sb.tile([2, J], f32, tag="wrt")
        nc.scalar.dma_start(wrt[:], wr_d.ap())

        psum1 = ps.tile([2, J], f32, tag="psum1")
        for c in range(NCH1):
            nc.tensor.matmul(
                psum1[:],
                x1t[:, c, :],
                w1tiles[c // CPB1][:, c % CPB1, :],
                start=(c == 0),
                stop=(c == NCH1 - 1),
            )
        psum2 = ps.tile([2, J], f32, tag="psum2")
        for c in range(NCH2):
            nc.tensor.matmul(
                psum2[:],
                x1t[:, c, :],
                w2tiles[c // CPB1][:, c % CPB1, :],
                start=(c == 0),
                stop=(c == NCH2 - 1),
            )

        pre1 = sb.tile([2, J], f32, tag="pre1")
        nc.vector.tensor_tensor(pre1[:], psum1[:], b1t[:], add)
        m = sb.tile([2, J], f32, tag="m")
        nc.scalar.activation(m[:], pre1[:], Tanh)
        s1 = sb.tile([2, J + 1], f32, tag="s1")
        nc.vector.tensor_copy(s1[:, 0:J], psum2[:])
        scr = sb.tile([2, J], f32, tag="scr")
        nc.vector.tensor_tensor(scr[:], m[:], wrt[:], mult)
        nc.vector.tensor_reduce(s1[:, J : J + 1], scr[:], mybir.AxisListType.X, add)
        nc.sync.dma_start(s1_d.ap(), s1[:])

    nc.compile()
    return nc


def _build_b():
    """Gates: alpha + la/ra from gathered partials, then sharded tree-LSTM."""
    from concourse import mybir

    f32 = mybir.dt.float32
    mdt = mybir.dt.bfloat16 if MM_DTYPE == "bf16" else mybir.dt.float32
    Tanh = mybir.ActivationFunctionType.Tanh
    Sigmoid = mybir.ActivationFunctionType.Sigmoid
    add = mybir.AluOpType.add
    mult = mybir.AluOpType.mult

    nc = _mk_nc()
    ag_d = nc.dram_tensor("ag", [R2, J + 1], f32, kind="ExternalInput")
    wg_d = nc.dram_tensor("wg", [NCHG, KP, 4 * J], mdt, kind="ExternalInput")
    brep_d = nc.dram_tensor("brep", [R2, J], f32, kind="ExternalInput")
    emask_d = nc.dram_tensor("emask", [R2, R2], f32, kind="ExternalInput")
    ident_d = nc.dram_tensor("ident", [R2, R2], mdt, kind="ExternalInput")
    bg_d = nc.dram_tensor("bg", [1, 4 * J], f32, kind="ExternalInput")
    lcrc_d = nc.dram_tensor("lcrc", [1, 2 * J], f32, kind="ExternalInput")
    out_d = nc.dram_tensor("out", [1, 2 * J], f32, kind="ExternalOutput")

    ctx, tc, sb, ps = _ctx_pools(nc)
    with ctx:
        # gate weights stream on sync ring, 1MB per DMA
        CPBG = 4
        wgtiles = []
        for b in range(NCHG // CPBG):
            t = sb.tile([KP, CPBG, 4 * J], mdt, tag=f"wg_{b}")
            nc.sync.dma_start(
                t[:], wg_d.ap()[b * CPBG : (b + 1) * CPBG].transpose([1, 0, 2])
            )
            wgtiles.append(t)
        # small inputs on scalar ring / gpsimd
        pg = sb.tile([R2, J], f32, tag="pg")
        nc.scalar.dma_start(pg[:], ag_d.ap()[:, 0:J])
        ebt = sb.tile([R2, R2], f32, tag="ebt")
        nc.gpsimd.dma_start(
            ebt[:], ag_d.ap()[:, J : J + 1].transpose([1, 0]).partition_broadcast(R2)
        )
        brept = sb.tile([R2, J], f32, tag="brept")
        nc.scalar.dma_start(brept[:], brep_d.ap())
        emaskt = sb.tile([R2, R2], f32, tag="emaskt")
        nc.scalar.dma_start(emaskt[:], emask_d.ap())
        identt = sb.tile([R2, R2], mdt, tag="identt")
        nc.scalar.dma_start(identt[:], ident_d.ap())
        bgt = sb.tile([1, 4 * J], f32, tag="bgt")
        nc.scalar.dma_start(bgt[:], bg_d.ap())
        lcrct = sb.tile([1, 2 * J], f32, tag="lcrct")
        nc.scalar.dma_start(lcrct[:], lcrc_d.ap())

        # ---- alphas on all 16 rows ----
        den = sb.tile([R2, 1], f32, tag="den")
        nc.vector.tensor_reduce(den[:], ebt[:], mybir.AxisListType.X, add)
        scr16 = sb.tile([R2, R2], f32, tag="scr16")
        esel = sb.tile([R2, 1], f32, tag="esel")
        nc.vector.tensor_tensor(scr16[:], ebt[:], emaskt[:], mult)
        nc.vector.tensor_reduce(esel[:], scr16[:], mybir.AxisListType.X, add)
        denr = sb.tile([R2, 1], f32, tag="denr")
        nc.vector.reciprocal(denr[:], den[:])
        alpha = sb.tile([R2, 1], f32, tag="alpha")
        nc.vector.tensor_tensor(alpha[:], esel[:], denr[:], mult)

        # ---- la/ra: tanh(alpha * p + ma_b), gathered [16, 256] layout ----
        actin = sb.tile([R2, J], f32, tag="actin")
        nc.vector.scalar_tensor_tensor(actin[:], pg[:], alpha[:], brept[:], mult, add)
        laR = sb.tile([R2, J], mdt, tag="laR")
        nc.scalar.activation(laR[:], actin[:], Tanh)

        # ---- PE transpose into stationary layout: T0/T1 [128, 16] ----
        Ts = []
        for h in range(2):
            pt = ps.tile([KP, R2], mdt, tag=f"psT{h}")
            nc.tensor.transpose(pt[:], laR[:, h * KP : (h + 1) * KP], identt[:])
            t = sb.tile([KP, R2], mdt, tag=f"T{h}")
            nc.vector.tensor_copy(t[:], pt[:])
            Ts.append(t)

        def xcat_col(c):
            cc = c % NCH2
            col = 2 * (cc // 2) + (0 if c < NCH2 else 1)
            return Ts[cc % 2][:, col : col + 1]

        # ---- gate matmuls: two N=512 accumulation groups ----
        psgA = ps.tile([1, 2 * J], f32, tag="psgA")
        psgB = ps.tile([1, 2 * J], f32, tag="psgB")
        for c in range(NCHG):
            lhs = xcat_col(c)
            wt = wgtiles[c // CPBG]
            nc.tensor.matmul(
                psgA[:], lhs, wt[:, c % CPBG, 0 : 2 * J],
                start=(c == 0), stop=(c == NCHG - 1),
            )
            nc.tensor.matmul(
                psgB[:], lhs, wt[:, c % CPBG, 2 * J : 4 * J],
                start=(c == 0), stop=(c == NCHG - 1),
            )

        # ---- gate tail ----
        zA = sb.tile([1, 2 * J], f32, tag="zA")
        nc.vector.tensor_tensor(zA[:], psgA[:], bgt[:, 0 : 2 * J], add)
        zB = sb.tile([1, 2 * J], f32, tag="zB")
        nc.vector.tensor_tensor(zB[:], psgB[:], bgt[:, 2 * J : 4 * J], add)
        sA = sb.tile([1, 2 * J], f32, tag="sA")  # [i | lf]
        nc.scalar.activation(sA[:], zA[:], Sigmoid)
        sB = sb.tile([1, 2 * J], f32, tag="sB")  # [rf | u]
        nc.scalar.activation(sB[:, 0:J], zB[:, 0:J], Sigmoid)
        nc.scalar.activation(sB[:, J : 2 * J], zB[:, J : 2 * J], Tanh)

        co = sb.tile([1, 2 * J], f32, tag="co")
        t1 = sb.tile([1, J], f32, tag="t1")
        nc.vector.tensor_tensor(t1[:], sA[:, 0:J], sB[:, J : 2 * J], mult)  # i*u
        t2 = sb.tile([1, J], f32, tag="t2")
        nc.vector.tensor_tensor(t2[:], sA[:, J : 2 * J], lcrct[:, 0:J], mult)
        t3 = sb.tile([1, J], f32, tag="t3")
        nc.vector.tensor_tensor(t3[:], sB[:, 0:J], lcrct[:, J : 2 * J], mult)
        t4 = sb.tile([1, J], f32, tag="t4")
        nc.vector.tensor_tensor(t4[:], t1[:], t2[:], add)
        nc.vector.tensor_tensor(co[:, 0:J], t4[:], t3[:], add)  # c
        nc.scalar.activation(co[:, J : 2 * J], co[:, 0:J], Tanh)  # h

        nc.sync.dma_start(out_d.ap(), co[:])

    nc.compile()
    return nc


def _shard_inputs(inp):
    mdt = _np_mm_dtype()
    f32 = np.float32

    def a(x):
        return np.asarray(x, dtype=np.float32)

    lh, rh, S = a(inp["lh"])[0], a(inp["rh"])[0], a(inp["S"])[0]
    lc, rc, w = a(inp["lc"])[0], a(inp["rc"])[0], a(inp["w"])[0]

    # stationary x spread: [128, 32, 2]; cat = [lh|S] col0, [rh|S] col1
    xl = np.concatenate([lh, S]).reshape(NCH1, KP).T
    xr = np.concatenate([rh, S]).reshape(NCH1, KP).T
    x1sp = np.stack([xl, xr], axis=-1).astype(mdt)

    emask = np.zeros((R2, R2), np.float32)
    for r in range(R2):
        emask[r, r % 2 :: 2] = 1.0
    ident = np.eye(R2, dtype=mdt)
    brep = np.repeat(a(inp["ma_b"]).reshape(NCORES, J), 2, axis=0).astype(f32)

    maps_a, maps_b = [], []
    for d in range(NCORES):
        sl = slice(d * J, (d + 1) * J)
        w1m = np.concatenate(
            [a(inp["Wh_w"])[sl].T, a(inp["Us_w"])[sl].T], axis=0
        ).reshape(NCH1, KP, J).astype(mdt)
        w2m = a(inp["ma_w"])[sl].T.reshape(NCH2, KP, J).copy().astype(mdt)
        gs = []
        for l, r in (("ilh", "irh"), ("lflh", "lfrh"), ("rflh", "rfrh"), ("ulh", "urh")):
            gs.append(
                np.concatenate([a(inp[l + "_w"])[sl].T, a(inp[r + "_w"])[sl].T], axis=0)
            )
        wg = (
            np.stack(gs, axis=1).reshape(2 * MEM, 4 * J).reshape(NCHG, KP, 4 * J)
        ).astype(mdt)
        b1 = np.tile((a(inp["Wh_b"]) + a(inp["Us_b"]))[sl], (2, 1)).astype(f32)
        wr = np.tile(w[sl], (2, 1)).astype(f32)
        bg = np.concatenate(
            [
                (a(inp["ilh_b"]) + a(inp["irh_b"]))[sl],
                (a(inp["lflh_b"]) + a(inp["lfrh_b"]))[sl],
                (a(inp["rflh_b"]) + a(inp["rfrh_b"]))[sl],
                (a(inp["ulh_b"]) + a(inp["urh_b"]))[sl],
            ]
        ).reshape(1, 4 * J).astype(f32)
        lcrc = np.concatenate([lc[sl], rc[sl]]).reshape(1, 2 * J).astype(f32)
        maps_a.append({"x1sp": x1sp, "w1m": w1m, "w2m": w2m, "b1": b1, "wr": wr})
        maps_b.append(
            {
                "wg": wg,
                "brep": brep,
                "emask": emask,
                "ident": ident,
                "bg": bg,
                "lcrc": lcrc,
            }
        )
    return maps_a, maps_b


def kernel(**inputs):
    global LAST_RESULTS
    _ensure_ntff_hook()
    from concourse.bass_utils import run_bass_kernel_spmd

    key = MM_DTYPE
    if key not in _COMPILED:
        _COMPILED[key] = (_build_a(), _build_b())
    nc_a, nc_b = _COMPILED[key]

    maps_a, maps_b = _shard_inputs(inputs)
    cores = list(range(NCORES))

    res_a = run_bass_kernel_spmd(nc_a, maps_a, cores)
    ag = np.concatenate(
        [res_a.results[d]["s1"] for d in range(NCORES)], axis=0
    ).astype(np.float32)  # [16, 257] -- pure gather, no host math
    for mb in maps_b:
        mb["ag"] = ag

    res_b = run_bass_kernel_spmd(nc_b, maps_b, cores)
    LAST_RESULTS = [res_a, res_b]

    c = np.concatenate([res_b.results[d]["out"][0, 0:J] for d in range(NCORES)])
    h = np.concatenate([res_b.results[d]["out"][0, J : 2 * J] for d in range(NCORES)])
    return (c.reshape(1, MEM).astype(np.float32), h.reshape(1, MEM).astype(np.float32))
